# revision 1
# baseline (speedup 1.0000x reference)
"""Trainium2 Bass kernel for nn_CE_RVQ: 8-layer residual VQ with CE loss.

Sharding: data-parallel over batch (16 batches -> 2 per core x 8 cores).
Weights/codebooks replicated. Final scalar loss assembled on host from
per-core per-partition partial sums (lse and picked terms).

Device algorithm per core, per layer i (tokens = 2*4096 = 8192, groups of 512):
  xpT[c,t]   = Win_i @ res  (PE, fp32r)  + b_in (ACT bias on PSUM->SBUF evac)
  g[t,k]     = rank1(ones x -e2/DC) + (2/DC) xpT.T @ embed_i.T  (PE, fp32r, PSUM)
  expg       = exp(g) (ACT, PSUM->SBUF; accum_out = sum_k exp -> lse), or copy
               for non-loss layers
  top8       = vector.max(expg)   (DVE)   -> m = top8[:,0]
  onehot     = (expg == m) fp16   (GPSIMD tensor_scalar is_equal)
  onehotT    = dma_start_transpose(onehot)  (DMA xbar, [k,t] chunked layout)
  quantT     = sum_kc e_kc.T @ onehotT_kc   (PE, fp16)
  qoutT      = rank1(b_out x ones) + WoutT.T @ quantT  (PE, fp32r)
  res       -= qoutT  (DVE tensor_tensor subtract, in place)
  picked     = TTR(xpT * EtgtT_hostgathered) summed per partition (DVE)
Loss = mean_l [ (sum lse - sum picked_x + sum e2[tgt]/DC) / (B*T) ].
"""

import os
import sys
import numpy as np

for _p in ("/opt/trn_rl_repo", "/opt/trn_rl_repo/concourse"):
    if _p not in sys.path:
        sys.path.insert(0, _p)

B, D, T = 16, 256, 4096
NQ, K, DC = 8, 1024, 128
SAMPLE_IDX = (0, 1, 2, 3)
N_CORES = 8
BPC = B // N_CORES          # batches per core
GROUP = 512                 # tokens per group
LAST_RESULTS = None         # BassKernelResults of the most recent run (for test.py)

_PROGRAM_CACHE = {}
VARIANT = "full"   # debug bisection knob: full | dve_eq | no_tr | dve_eq_no_tr


def build_program(tokens=BPC * T):
    """Build the SPMD Bass program for one core handling `tokens` tokens."""
    import concourse.bass as bass
    import concourse.bacc as bacc
    import concourse.mybir as mybir
    import concourse.tile as tile

    f32, f32r, f16 = mybir.dt.float32, mybir.dt.float32r, mybir.dt.float16
    AF = mybir.ActivationFunctionType
    ALU = mybir.AluOpType
    AX = mybir.AxisListType

    n_groups = tokens // GROUP
    TPG = GROUP // 128                     # token tiles per group
    NL = len(SAMPLE_IDX)

    nc = bacc.Bacc("TRN2", target_bir_lowering=False, debug=False)

    def din(name, shape, dt=f32):
        return nc.dram_tensor(name, list(shape), dt, kind="ExternalInput").ap()

    resh_d = din("resh", (2, 128, tokens), f32r)            # residuals, d-chunked
    etgt_d = din("etgt", (NL, 128, tokens), f16)      # gathered embed[tgt].T
    win_d = din("win", (128, NQ * 2 * 128), f32r)           # WinT chunks [d,c]
    b_in_d = din("b_in", (128, NQ))
    eT2s_d = din("eT2s", (128, NQ * K), f32r)               # embed.T * 2/DC
    e2neg_d = din("e2neg", (1, NQ * K), f32r)               # -(e^2).sum/DC
    ef16_d = din("ef16", (128, NQ * K), f16)          # e chunks [k,c] fp16
    woT_d = din("woT", (128, NQ * D), f32r)                 # Wout.T [c,d]
    bout_d = din("bout", (1, NQ * D), f32r)
    ones_d = din("ones", (1, GROUP), f32r)
    out_d = nc.dram_tensor("loss_parts", [128, 2 * NL], f32,
                           kind="ExternalOutput").ap()

    with tile.TileContext(nc) as tc:
        with (
            tc.tile_pool(name="cpool", bufs=1) as cpool,
            tc.tile_pool(name="wpool", bufs=2) as wpool,
            tc.tile_pool(name="one", bufs=1) as onepool,
            tc.tile_pool(name="pxp", bufs=1, space="PSUM") as pxp,
            tc.tile_pool(name="pg", bufs=2, space="PSUM") as pg,
            tc.tile_pool(name="pq", bufs=1, space="PSUM") as pq,
            tc.tile_pool(name="pqo", bufs=1, space="PSUM") as pqo,
            tc.tile_pool(name="dpool", bufs=2, space="DRAM") as dpool,
        ):
            # ---- persistent tiles -------------------------------------------
            res_sb = [cpool.tile([128, tokens], f32r, tag=f"res{dc}", name=f"res{dc}")
                      for dc in range(2)]
            b_in_sb = cpool.tile([128, NQ], f32, tag="b_in", name="b_in")
            ones_sb = cpool.tile([1, GROUP], f32r, tag="ones", name="ones")
            acc_sb = cpool.tile([128, 2 * NL], f32, tag="acc", name="acc")

            for dc in range(2):
                nc.sync.dma_start(res_sb[dc][:], resh_d[dc])
            nc.sync.dma_start(b_in_sb[:], b_in_d)
            nc.sync.dma_start(ones_sb[:], ones_d)

            for i in range(NQ):
                is_loss = i in SAMPLE_IDX
                li = SAMPLE_IDX.index(i) if is_loss else -1
                # per-layer weights, double-buffered so next layer prefetches
                win_l = wpool.tile([128, 2 * 128], f32r, tag="win_l", name="win_l")
                nc.sync.dma_start(win_l[:], win_d[:, i * 256:(i + 1) * 256])
                eT2s_l = wpool.tile([128, K], f32r, tag="eT2s_l", name="eT2s_l")
                nc.sync.dma_start(eT2s_l[:], eT2s_d[:, i * K:(i + 1) * K])
                ef16_l = wpool.tile([128, K], f16, tag="ef16_l", name="ef16_l")
                nc.sync.dma_start(ef16_l[:], ef16_d[:, i * K:(i + 1) * K])
                woT_l = wpool.tile([128, D], f32r, tag="woT_l", name="woT_l")
                nc.sync.dma_start(woT_l[:], woT_d[:, i * D:(i + 1) * D])
                e2neg_l = wpool.tile([1, K], f32r, tag="e2neg_l", name="e2neg_l")
                nc.sync.dma_start(e2neg_l[:], e2neg_d[:, i * K:(i + 1) * K])
                bout_l = wpool.tile([1, D], f32r, tag="bout_l", name="bout_l")
                nc.sync.dma_start(bout_l[:], bout_d[:, i * D:(i + 1) * D])
                if is_loss:
                    etgt_sb = onepool.tile([128, tokens], f16, tag="etgt", name="etgt")
                    nc.sync.dma_start(etgt_sb[:], etgt_d[li])
                    s_cols = wpool.tile([128, n_groups * TPG], f32, tag="scols", name="scols")
                    ttr_cols = wpool.tile([128, n_groups], f32, tag="ttrcols", name="ttrcols")

                for g in range(n_groups):
                    gsl = slice(g * GROUP, (g + 1) * GROUP)
                    # ---- project in: xpT [c, t] -----------------------------
                    xp_ps = pxp.tile([128, GROUP], f32, tag="xp", name="xp")
                    for dc in range(2):
                        nc.tensor.matmul(
                            xp_ps[:],
                            lhsT=win_l[:, dc * 128:(dc + 1) * 128],
                            rhs=res_sb[dc][:, gsl],
                            start=(dc == 0), stop=(dc == 1))
                    xp_sb = wpool.tile([128, GROUP], f32r, tag="xp_sb", name="xp_sb")
                    nc.scalar.activation(xp_sb[:], xp_ps[:], AF.Identity,
                                         bias=b_in_sb[:, i:i + 1])

                    # ---- distances + argmax-onehot per 128-token tile -------
                    ohs = dpool.tile([GROUP, K], f16, tag="ohs", name="ohs", bufs=3)
                    ohT = onepool.tile([128, 8, GROUP], f16, tag="ohT", name="ohT", bufs=2)
                    for j in range(TPG):
                        t0 = j * 128
                        g_ps = pg.tile([128, K], f32, tag="g", name="g")
                        for kh in range(2):
                            ksl = slice(kh * 512, (kh + 1) * 512)
                            nc.tensor.matmul(
                                g_ps[:, ksl],
                                lhsT=ones_sb[:, 0:128],
                                rhs=e2neg_l[:, kh * 512:(kh + 1) * 512],
                                start=True, stop=False)
                            nc.tensor.matmul(
                                g_ps[:, ksl],
                                lhsT=xp_sb[:, t0:t0 + 128],
                                rhs=eT2s_l[:, ksl],
                                start=False, stop=True)
                        expg = wpool.tile([128, K], f32, tag="expg", name="expg", bufs=3)
                        if is_loss:
                            nc.scalar.activation(
                                expg[:], g_ps[:], AF.Exp,
                                accum_out=s_cols[:, g * TPG + j:g * TPG + j + 1])
                        else:
                            nc.scalar.activation(expg[:], g_ps[:], AF.Copy)
                        top8 = wpool.tile([128, 8], f32, tag="top8", name="top8")
                        nc.vector.max(top8[:], expg[:])
                        oh = wpool.tile([128, K], f16, tag="oh", name="oh", bufs=3)
                        if VARIANT in ("dve_eq", "dve_eq_no_tr"):
                            nc.vector.tensor_scalar(oh[:], expg[:],
                                                    top8[:, 0:1], None,
                                                    ALU.is_equal)
                        else:
                            nc.gpsimd.tensor_scalar(oh[:], expg[:],
                                                    top8[:, 0:1], None,
                                                    ALU.is_equal)
                        nc.sync.dma_start(ohs[t0:t0 + 128, :], oh[:])

                    nc.sync.dma_start_transpose(out=ohT[:], in_=ohs[:])

                    # ---- quantize: quantT [c, t] = sum_kc e_kc.T @ ohT_kc ---
                    q_ps = pq.tile([128, GROUP], f32, tag="q", name="q")
                    for kc in range(8):
                        nc.tensor.matmul(
                            q_ps[:],
                            lhsT=ef16_l[:, kc * 128:(kc + 1) * 128],
                            rhs=ohT[:, kc, :],
                            start=(kc == 0), stop=(kc == 7))
                    q_sb = wpool.tile([128, GROUP], f32r, tag="q_sb", name="q_sb")
                    nc.scalar.copy(q_sb[:], q_ps[:])

                    # ---- project out + residual update ----------------------
                    qo_ps = pqo.tile([128, 2 * GROUP], f32, tag="qo", name="qo")
                    for dh in range(2):
                        osl = slice(dh * GROUP, (dh + 1) * GROUP)
                        nc.tensor.matmul(
                            qo_ps[:, osl],
                            lhsT=bout_l[:, dh * 128:(dh + 1) * 128],
                            rhs=ones_sb[:],
                            start=True, stop=False)
                        nc.tensor.matmul(
                            qo_ps[:, osl],
                            lhsT=woT_l[:, dh * 128:(dh + 1) * 128],
                            rhs=q_sb[:],
                            start=False, stop=True)
                        nc.vector.tensor_tensor(
                            res_sb[dh][:, gsl], res_sb[dh][:, gsl],
                            qo_ps[:, osl], op=ALU.subtract)

                    # ---- picked term (loss layers) --------------------------
                    if is_loss:
                        etgt32 = wpool.tile([128, GROUP], f32, tag="etgt32", name="etgt32")
                        nc.scalar.copy(etgt32[:], etgt_sb[:, gsl])
                        junk = wpool.tile([128, GROUP], f32, tag="junk", name="junk")
                        nc.vector.tensor_mul(junk[:], xp_sb[:].bitcast(f32),
                                             etgt32[:])
                        nc.vector.tensor_reduce(ttr_cols[:, g:g + 1], junk[:],
                                                axis=AX.X, op=ALU.add)

                if is_loss:
                    lse_cols = wpool.tile([128, n_groups * TPG], f32,
                                          tag="lsecols", name="lsecols")
                    nc.scalar.activation(lse_cols[:], s_cols[:], AF.Ln)
                    nc.vector.tensor_reduce(acc_sb[:, li:li + 1], lse_cols[:],
                                            axis=AX.X, op=ALU.add)
                    nc.vector.tensor_reduce(acc_sb[:, NL + li:NL + li + 1],
                                            ttr_cols[:], axis=AX.X, op=ALU.add)

            nc.sync.dma_start(out_d, acc_sb[:])

    nc.compile()
    return nc


def prepare_inputs(diffusion_starts, target_latent_codes, Win, b_in, Wout,
                   b_out, embed, tokens=BPC * T):
    """Host-side sharding/layout prep. Returns (in_maps, e2tgt_sums)."""
    ds = np.ascontiguousarray(np.asarray(diffusion_starts, dtype=np.float32))
    tgt = np.asarray(target_latent_codes)
    Win = np.asarray(Win, dtype=np.float32)
    b_in = np.asarray(b_in, dtype=np.float32)
    Wout = np.asarray(Wout, dtype=np.float32)
    b_out = np.asarray(b_out, dtype=np.float32)
    embed = np.asarray(embed, dtype=np.float32)

    NL = len(SAMPLE_IDX)
    Tc = tokens // BPC                    # tokens per batch used

    win_flat = np.empty((128, NQ * 2 * 128), np.float32)
    eT2s_flat = np.empty((128, NQ * K), np.float32)
    ef16_flat = np.empty((128, NQ * K), np.float16)
    woT_flat = np.empty((128, NQ * D), np.float32)
    for i in range(NQ):
        wt = Win[i].T                     # [D, DC]
        for dc in range(2):
            win_flat[:, (i * 2 + dc) * 128:(i * 2 + dc + 1) * 128] = \
                wt[dc * 128:(dc + 1) * 128, :]
        eT2s_flat[:, i * K:(i + 1) * K] = embed[i].T * np.float32(2.0 / DC)
        for kc in range(8):
            ef16_flat[:, i * K + kc * 128:i * K + (kc + 1) * 128] = \
                embed[i][kc * 128:(kc + 1) * 128, :].astype(np.float16)
        woT_flat[:, i * D:(i + 1) * D] = Wout[i].T
    b_in_flat = np.ascontiguousarray(b_in.T)                      # [128, NQ]
    e2neg_flat = (-(embed.astype(np.float64) ** 2).sum(-1) / DC) \
        .astype(np.float32).reshape(1, NQ * K)
    bout_flat = b_out.reshape(1, NQ * D)
    ones_row = np.ones((1, GROUP), np.float32)
    e2 = (embed.astype(np.float64) ** 2).sum(-1) / DC             # [NQ, K]

    in_maps, e2tgt_sums = [], np.zeros((N_CORES, NL), np.float64)
    for c in range(N_CORES):
        resh = np.empty((2, 128, tokens), np.float32)
        etgt = np.empty((NL, 128, tokens), np.float16)
        for b in range(BPC):
            bb = c * BPC + b
            for dc in range(2):
                resh[dc, :, b * Tc:(b + 1) * Tc] = \
                    ds[bb, dc * 128:(dc + 1) * 128, :Tc]
            for li, i in enumerate(SAMPLE_IDX):
                ti = tgt[bb, i, :Tc].astype(np.int64)
                etgt[li, :, b * Tc:(b + 1) * Tc] = \
                    embed[i][ti].T.astype(np.float16)
                e2tgt_sums[c, li] += e2[i][ti].sum()
        in_maps.append({
            "resh": resh, "etgt": etgt, "win": win_flat, "b_in": b_in_flat,
            "eT2s": eT2s_flat, "e2neg": e2neg_flat, "ef16": ef16_flat,
            "woT": woT_flat, "bout": bout_flat, "ones": ones_row,
        })
    return in_maps, e2tgt_sums


def assemble_loss(results, e2tgt_sums, tokens=BPC * T):
    """results: list of per-core dicts with 'loss_parts' [128, 2*NL]."""
    NL = len(SAMPLE_IDX)
    n_tok = N_CORES * tokens
    losses = []
    for li in range(NL):
        s_lse = sum(float(r["loss_parts"][:, li].astype(np.float64).sum())
                    for r in results)
        s_ttr = sum(float(r["loss_parts"][:, NL + li].astype(np.float64).sum())
                    for r in results) * (2.0 / DC)
        s_e2 = float(e2tgt_sums[:, li].sum())
        losses.append((s_lse - s_ttr + s_e2) / n_tok)
    return np.float32(np.mean(losses))


def kernel(diffusion_starts, target_latent_codes, Win, b_in, Wout, b_out,
           embed):
    global LAST_RESULTS
    from concourse import bass_utils

    tokens = BPC * T
    if tokens not in _PROGRAM_CACHE:
        _PROGRAM_CACHE[tokens] = build_program(tokens)
    nc = _PROGRAM_CACHE[tokens]

    in_maps, e2tgt_sums = prepare_inputs(
        diffusion_starts, target_latent_codes, Win, b_in, Wout, b_out, embed,
        tokens)
    LAST_RESULTS = bass_utils.run_bass_kernel_spmd(
        nc, in_maps, core_ids=list(range(N_CORES)),
        trace=os.environ.get("KERNEL_TRACE", "") == "1")
    return assemble_loss(LAST_RESULTS.results, e2tgt_sums, tokens)



# revision 43
# speedup vs baseline: 4.2148x; 4.2148x over previous
"""Trainium2 Bass kernel for nn_CE_RVQ: 8-layer residual VQ with CE loss.

Only layers 0-3 contribute to the output (SAMPLE_IDX covers 0-3 and the
residual stream past layer 3 is never read), so layers 4-7 are skipped and
layer 3 computes logits/loss terms only.

Sharding: data-parallel over batch (16 batches -> 2 per core x 8 cores).
Weights/codebooks replicated. Scalar loss assembled on host from per-core
per-partition partial sums.

Device algorithm per core (tokens = 2*4096 = 8192). The residual stream is
never materialized past layer 0: with q_i = e_i[argmin] the projections
satisfy  xp_j = Win_j res0 - sum_{i<j} (Win_j Wout_i) q_i - bias terms,
so later layers correct their projection with small [DC,DC] matmuls instead
of updating res (no DVE subtract passes, no gather).

Per layer i, per 512-token slab:
  xpT[c,t]  = WinT_i.T @ res0 - sum_{ii<i} negMT_(ii,i).T @ q16_ii  (PE, PSUM)
  xp16      = xpT + bias_i      (DVE/ACT evac, fp16)
  per 128-token dist tile:
    g[t,k]  = rank1(ones x e2neg) + xp_tile.T @ eT2s   (PE fp16, PSUM fp32)
    expg    = exp(g) fp16, accum_out -> lse partial    (ACT; some tiles sum
              on DVE instead to balance engines)
    m       = max(expg)       (DVE: TT-max halves then vector.max)
    oh      = (expg == m)     (DVE tensor_scalar is_equal, fp16, 4x mode)
    ohT     = dma_start_transpose(oh) SBUF->SBUF into [k-chunks, t]
  qT[c,t]   = sum_kc ef16_kc.T @ ohT_kc   (PE fp16, 1-slab deferred) -> q16
picked     = sum_t (xpT * EtgtT) * 2/DC   (DVE mult + reduce-accum)
Loss_l = (sum lse - sum picked + sum e2[tgt]/DC) / (B*T), mean over 4.
"""

import os
import sys
import numpy as np

for _p in ("/opt/trn_rl_repo", "/opt/trn_rl_repo/concourse"):
    if _p not in sys.path:
        sys.path.insert(0, _p)

B, D, T = 16, 256, 4096
NQ, K, DC = 8, 1024, 128
SAMPLE_IDX = (0, 1, 2, 3)
NL = len(SAMPLE_IDX)            # effective layers (4..7 are dead code)
N_CORES = 8
BPC = B // N_CORES              # batches per core
GROUP = 2048                    # tokens per xp/picked group
SLAB = 512                      # tokens per oh-transpose / q matmul slab
LAST_RESULTS = None

_PROGRAM_CACHE = {}

# tuning knobs
LSE_DVE_FRAC = (1, 12)  # dist tiles with j%12<1 sum exp on DVE not ACT
ISEQ_POOL_MOD = 8       # dist tiles with j%8==7 run is_equal on Pool

_PAIRS = [(ii, i) for i in range(NL) for ii in range(i)]  # correction pairs


def build_program(tokens=BPC * T):
    import concourse.bass as bass
    import concourse.bacc as bacc
    import concourse.mybir as mybir
    import concourse.tile as tile

    f32, f16, bf16 = mybir.dt.float32, mybir.dt.float16, mybir.dt.bfloat16
    AF = mybir.ActivationFunctionType
    ALU = mybir.AluOpType
    AX = mybir.AxisListType

    n_groups = tokens // GROUP
    n_tiles = tokens // 128                 # lse columns per layer
    TPG = GROUP // 128                      # dist tiles per group (16)

    nc = bacc.Bacc("TRN2", target_bir_lowering=False, debug=False)

    def din(name, shape, dt=f32):
        return nc.dram_tensor(name, list(shape), dt, kind="ExternalInput").ap()

    resh_d = din("resh", (2, 128, tokens), bf16)       # residuals, d-chunked
    etgt_d = din("etgt", (NL, 128, tokens), f16)       # embed[tgt].T per layer
    win_d = din("win", (128, NL * 2 * 128), bf16)      # WinT chunks [d,c]
    bias_d = din("bias", (128, NL))                    # b_in - sum Win b_out
    eT2s_d = din("eT2s", (128, NL * K), f16)           # embed.T * 2/DC
    e2neg_d = din("e2neg", (1, NL * K), f16)           # -(e^2).sum/DC
    ef16_d = din("ef16", (128, (NL - 1) * K), f16)     # e k-chunks [k,c]
    negmt_d = din("negmt", (128, len(_PAIRS) * 128), f16)  # -(Win_i Wout_ii).T
    ones_d = din("ones", (1, 128), f16)
    out_d = nc.dram_tensor("loss_parts", [128, 2 * NL], f32,
                           kind="ExternalOutput").ap()

    with tile.TileContext(nc) as tc:
        with (
            tc.tile_pool(name="cpool", bufs=1) as cpool,
            tc.tile_pool(name="xpool", bufs=2) as xpool,
            tc.tile_pool(name="epool", bufs=2) as epool,
            tc.tile_pool(name="spool", bufs=6) as spool,
            tc.tile_pool(name="tpool", bufs=3) as tpool,
            tc.tile_pool(name="pxq", bufs=2, space="PSUM") as pxq,
            tc.tile_pool(name="pg", bufs=2, space="PSUM") as pg,
            tc.tile_pool(name="pq2", bufs=2, space="PSUM") as pq2,
        ):
            # ---- persistent tiles ------------------------------------------
            res_sb = [cpool.tile([128, tokens], bf16, tag=f"res{dc}",
                                 name=f"res{dc}") for dc in range(2)]
            win_sb = cpool.tile([128, NL * 256], bf16, tag="win", name="win")
            bias_sb = cpool.tile([128, NL], f32, tag="bias", name="bias")
            eT2s_sb = cpool.tile([128, NL * K], f16, tag="eT2s", name="eT2s")
            e2neg_sb = cpool.tile([1, NL * K], f16, tag="e2neg", name="e2neg")
            ef16_sb = cpool.tile([128, (NL - 1) * K], f16, tag="ef16",
                                 name="ef16")
            negmt_sb = cpool.tile([128, len(_PAIRS) * 128], f16, tag="negmt",
                                  name="negmt")
            ones_sb = cpool.tile([1, 128], f16, tag="ones", name="ones")
            acc_sb = cpool.tile([128, 2 * NL], f32, tag="acc", name="acc")
            s_cols = cpool.tile([128, NL * n_tiles], f32, tag="scols",
                                name="scols")
            ttr_cols = cpool.tile([128, NL * n_groups], f32, tag="ttr",
                                  name="ttr")
            # q16[i]: quantized embeddings for layer i, all tokens (fp16)
            q16 = [cpool.tile([128, tokens], f16, tag=f"q16_{i}",
                              name=f"q16_{i}") for i in range(NL - 1)]

            for dc in range(2):
                nc.sync.dma_start(res_sb[dc][:], resh_d[dc])
            nc.sync.dma_start(win_sb[:], win_d)
            nc.sync.dma_start(bias_sb[:], bias_d)
            nc.sync.dma_start(eT2s_sb[:], eT2s_d)
            nc.sync.dma_start(e2neg_sb[:], e2neg_d)
            nc.sync.dma_start(ef16_sb[:], ef16_d)
            nc.sync.dma_start(negmt_sb[:], negmt_d)
            nc.sync.dma_start(ones_sb[:], ones_d)

            for i in range(NL):
                pending = []
                for g in range(n_groups):
                    base = g * GROUP
                    gsl = slice(base, base + GROUP)

                    # ---- project in + corrections: xpT [c, t] fp16 ----------
                    xp16 = xpool.tile([128, GROUP], f16, tag="xp16",
                                      name="xp16")
                    for s in range(GROUP // SLAB):
                        ssl = slice(base + s * SLAB, base + (s + 1) * SLAB)
                        lsl = slice(s * SLAB, (s + 1) * SLAB)
                        xp_ps = pxq.tile([128, SLAB], f32, tag="xpq",
                                         name="xp")
                        for dc in range(2):
                            nc.tensor.matmul(
                                xp_ps[:],
                                lhsT=win_sb[:, (i * 2 + dc) * 128:
                                            (i * 2 + dc + 1) * 128],
                                rhs=res_sb[dc][:, ssl],
                                start=(dc == 0), stop=(dc == 1 and i == 0))
                        for ii in range(i):
                            p = _PAIRS.index((ii, i))
                            nc.tensor.matmul(
                                xp_ps[:],
                                lhsT=negmt_sb[:, p * 128:(p + 1) * 128],
                                rhs=q16[ii][:, ssl],
                                start=False, stop=(ii == i - 1))
                        if (g + s) % 2 == 0:
                            nc.scalar.activation(
                                xp16[:, lsl], xp_ps[:], AF.Identity,
                                bias=bias_sb[:, i:i + 1])
                        else:
                            nc.vector.tensor_scalar(
                                xp16[:, lsl], xp_ps[:],
                                bias_sb[:, i:i + 1], None, ALU.add)

                    # ---- picked: sum_t xp*etgt * 2/DC -----------------------
                    etgt_g = epool.tile([128, GROUP], f16, tag="etgt",
                                        name="etgt")
                    nc.sync.dma_start(etgt_g[:], etgt_d[i][:, gsl])
                    junk2k = epool.tile([128, GROUP], f16, tag="junk2k",
                                        name="junk2k")
                    nc.vector.tensor_tensor(junk2k[:], xp16[:], etgt_g[:],
                                            op=ALU.mult)
                    junk2b = epool.tile([128, GROUP], f16, tag="junk2b",
                                        name="junk2b")
                    nc.vector.tensor_scalar(
                        junk2b[:], junk2k[:], float(2.0 / DC), 0.0,
                        ALU.mult, ALU.add,
                        accum_out=ttr_cols[:, i * n_groups + g:
                                           i * n_groups + g + 1])

                    # ---- distance tiles + onehot ---------------------------
                    def emit_qmm(s, gbase, ohT):
                        # quantize matmul for slab s (emitted ~2 slabs late so
                        # the oh DMA-transpose latency hides behind dist work)
                        q_ps = pq2.tile([128, SLAB], f32, tag="q", name="q")
                        for kc in range(8):
                            nc.tensor.matmul(
                                q_ps[:],
                                lhsT=ef16_sb[:, i * K + kc * 128:
                                             i * K + (kc + 1) * 128],
                                rhs=ohT[:, kc, :],
                                start=(kc == 0), stop=(kc == 7))
                        if s % 3 == 0:
                            nc.scalar.copy(
                                q16[i][:, gbase + s * SLAB:
                                       gbase + (s + 1) * SLAB], q_ps[:])
                        else:
                            nc.vector.tensor_copy(
                                q16[i][:, gbase + s * SLAB:
                                       gbase + (s + 1) * SLAB], q_ps[:])

                    for s in range(GROUP // SLAB):
                        ohT = (tpool.tile([128, 8, SLAB], f16, tag="ohT",
                                          name="ohT")
                               if i < NL - 1 else None)
                        for jj in range(SLAB // 128):
                            j = s * (SLAB // 128) + jj
                            col = i * n_tiles + g * TPG + j
                            g_ps = pg.tile([128, K], f32, tag="g", name="g")
                            lhsT = xp16[:, j * 128:(j + 1) * 128]
                            # both rank1s first: one ones-lhsT load, one xp load
                            for kh in range(2):
                                khs = slice(kh * 512, (kh + 1) * 512)
                                nc.tensor.matmul(
                                    g_ps[:, khs],
                                    lhsT=ones_sb[:, 0:128],
                                    rhs=e2neg_sb[:, i * K + kh * 512:
                                                 i * K + (kh + 1) * 512],
                                    start=True, stop=False)
                            for kh in range(2):
                                khs = slice(kh * 512, (kh + 1) * 512)
                                nc.tensor.matmul(
                                    g_ps[:, khs],
                                    lhsT=lhsT,
                                    rhs=eT2s_sb[:, i * K + kh * 512:
                                                i * K + (kh + 1) * 512],
                                    start=False, stop=True)
                            expg = spool.tile([128, K], f16, tag="expg",
                                              name="expg")
                            if j % LSE_DVE_FRAC[1] < LSE_DVE_FRAC[0]:
                                # balance: sum exp on DVE via bypass+accum
                                nc.scalar.activation(expg[:], g_ps[:], AF.Exp)
                                junk1k = spool.tile([128, K], f16,
                                                    tag="junk1k",
                                                    name="junk1k")
                                nc.vector.tensor_scalar(
                                    junk1k[:], expg[:], 1.0, 0.0, ALU.mult,
                                    ALU.add,
                                    accum_out=s_cols[:, col:col + 1])
                            else:
                                nc.scalar.activation(
                                    expg[:], g_ps[:], AF.Exp,
                                    accum_out=s_cols[:, col:col + 1])
                            if i < NL - 1:
                                half = spool.tile([128, K // 2], f16,
                                                  tag="half", name="half")
                                nc.vector.tensor_tensor(
                                    half[:], expg[:, :K // 2],
                                    expg[:, K // 2:], op=ALU.max)
                                top8 = spool.tile([128, 8], f32, tag="top8",
                                                  name="top8")
                                nc.vector.max(top8[:], half[:])
                                oh = spool.tile([128, K], f16, tag="oh",
                                                name="oh")
                                iseq_eng = (nc.gpsimd
                                            if j % ISEQ_POOL_MOD
                                            == ISEQ_POOL_MOD - 1
                                            else nc.vector)
                                iseq_eng.tensor_scalar(
                                    oh[:], expg[:], top8[:, 0:1], None,
                                    ALU.is_equal)
                                # SBUF->SBUF XBAR transpose straight into the
                                # slab's [k,t] buffer (no DRAM bounce)
                                nc.sync.dma_start_transpose(
                                    out=ohT[:, :, jj * 128:(jj + 1) * 128],
                                    in_=oh[:])

                        # ---- defer quantize matmul -------------------------
                        if i < NL - 1:
                            pending.append((s, base, ohT))
                            if len(pending) > 2:
                                emit_qmm(*pending.pop(0))
                if i < NL - 1:
                    for item in pending:
                        emit_qmm(*item)
                    pending = []

            # ---- loss partials ---------------------------------------------
            for i in range(NL):
                lse_cols = epool.tile([128, n_tiles], f32, tag="lse",
                                      name="lse")
                nc.scalar.activation(
                    lse_cols[:], s_cols[:, i * n_tiles:(i + 1) * n_tiles],
                    AF.Ln)
                nc.vector.tensor_reduce(
                    acc_sb[:, i:i + 1], lse_cols[:], axis=AX.X, op=ALU.add)
                nc.vector.tensor_reduce(
                    acc_sb[:, NL + i:NL + i + 1],
                    ttr_cols[:, i * n_groups:(i + 1) * n_groups],
                    axis=AX.X, op=ALU.add)

            nc.sync.dma_start(out_d, acc_sb[:])

    nc.compile()
    return nc


def prepare_inputs(diffusion_starts, target_latent_codes, Win, b_in, Wout,
                   b_out, embed, tokens=BPC * T):
    """Host-side sharding/layout prep. Returns (in_maps, e2tgt_sums)."""
    ds = np.ascontiguousarray(np.asarray(diffusion_starts, dtype=np.float32))
    tgt = np.asarray(target_latent_codes)
    Win = np.asarray(Win, dtype=np.float32)
    b_in = np.asarray(b_in, dtype=np.float32)
    Wout = np.asarray(Wout, dtype=np.float32)
    b_out = np.asarray(b_out, dtype=np.float32)
    embed = np.asarray(embed, dtype=np.float32)

    Tc = tokens // BPC

    win_flat = np.empty((128, NL * 256), np.float32)
    for i in range(NL):
        wt = Win[i].T                     # [D, DC]
        for dc in range(2):
            win_flat[:, (i * 2 + dc) * 128:(i * 2 + dc + 1) * 128] = \
                wt[dc * 128:(dc + 1) * 128, :]
    eT2s_flat = np.concatenate(
        [embed[i].T * np.float32(2.0 / DC) for i in range(NL)], axis=1)
    e2 = (embed.astype(np.float64) ** 2).sum(-1) / DC          # [NQ, K]
    e2neg_flat = (-e2[:NL]).astype(np.float32).reshape(1, NL * K)
    ef16_flat = np.empty((128, (NL - 1) * K), np.float16)
    for i in range(NL - 1):
        for kc in range(8):
            ef16_flat[:, i * K + kc * 128:i * K + (kc + 1) * 128] = \
                embed[i][kc * 128:(kc + 1) * 128, :].astype(np.float16)
    negmt_flat = np.empty((128, len(_PAIRS) * 128), np.float32)
    for p, (ii, i) in enumerate(_PAIRS):
        negmt_flat[:, p * 128:(p + 1) * 128] = -(Win[i] @ Wout[ii]).T
    bias_flat = np.empty((128, NL), np.float32)
    for i in range(NL):
        bias_flat[:, i] = b_in[i] - sum(
            Win[i] @ b_out[ii] for ii in range(i)) if i else b_in[i]
    ones_row = np.ones((1, 128), np.float16)

    in_maps, e2tgt_sums = [], np.zeros((N_CORES, NL), np.float64)
    for c in range(N_CORES):
        resh = np.empty((2, 128, tokens), np.float32)
        etgt = np.empty((NL, 128, tokens), np.float16)
        for b in range(BPC):
            bb = c * BPC + b
            for dc in range(2):
                resh[dc, :, b * Tc:(b + 1) * Tc] = \
                    ds[bb, dc * 128:(dc + 1) * 128, :Tc]
            for li, i in enumerate(SAMPLE_IDX):
                ti = tgt[bb, i, :Tc].astype(np.int64)
                etgt[li, :, b * Tc:(b + 1) * Tc] = \
                    embed[i][ti].T.astype(np.float16)
                e2tgt_sums[c, li] += e2[i][ti].sum()
        in_maps.append({
            "resh": resh.astype(_BF16),
            "etgt": etgt, "win": win_flat.astype(_BF16),
            "bias": bias_flat, "eT2s": eT2s_flat.astype(np.float16),
            "e2neg": e2neg_flat.astype(np.float16),
            "ef16": ef16_flat,
            "negmt": negmt_flat.astype(np.float16),
            "ones": ones_row,
        })
    return in_maps, e2tgt_sums


try:
    import ml_dtypes
    _BF16 = ml_dtypes.bfloat16
except ImportError:      # pragma: no cover
    _BF16 = np.float32


def assemble_loss(results, e2tgt_sums, tokens=BPC * T):
    """results: list of per-core dicts with 'loss_parts' [128, 2*NL]."""
    n_tok = N_CORES * tokens
    losses = []
    for li in range(NL):
        s_lse = sum(float(r["loss_parts"][:, li].astype(np.float64).sum())
                    for r in results)
        s_ttr = sum(float(r["loss_parts"][:, NL + li].astype(np.float64).sum())
                    for r in results)
        s_e2 = float(e2tgt_sums[:, li].sum())
        losses.append((s_lse - s_ttr + s_e2) / n_tok)
    return np.float32(np.mean(losses))


def kernel(diffusion_starts, target_latent_codes, Win, b_in, Wout, b_out,
           embed):
    global LAST_RESULTS
    from concourse import bass_utils

    tokens = BPC * T
    if tokens not in _PROGRAM_CACHE:
        _PROGRAM_CACHE[tokens] = build_program(tokens)
    nc = _PROGRAM_CACHE[tokens]

    in_maps, e2tgt_sums = prepare_inputs(
        diffusion_starts, target_latent_codes, Win, b_in, Wout, b_out, embed,
        tokens)
    LAST_RESULTS = bass_utils.run_bass_kernel_spmd(
        nc, in_maps, core_ids=list(range(N_CORES)),
        trace=os.environ.get("KERNEL_TRACE", "") == "1")
    return assemble_loss(LAST_RESULTS.results, e2tgt_sums, tokens)
